# revision 28
# baseline (speedup 1.0000x reference)
"""Trainium2 Bass kernel for degree-3 uniform B-spline basis evaluation.

Problem: x (1024, 8192) fp32, knots = linspace(-2, 2, 12) -> out (1024, 8192, 8)
where out[..., i] is the i-th cubic B-spline basis function (Cox-de Boor).

Math. With uniform knots (spacing h), basis i is a shifted cardinal cubic
B-spline: out_i(x) = C((x - knots[0])/h - i), C supported on [0, 4). Writing
a = |(x - knots[0])/h - i - 2| (distance to the support center), C reflects to

    C = relu(2 - a)^3 / 6  -  (2/3) * relu(1 - a)^3

which is numerically clean and returns exact zeros outside the support.

Kernel design (measured-HW engine balance at the DVE write floor):

Every output element must be produced by one DVE write: the fused cubic
needs 8 ALU slices, the DVE pipeline max, so the terminal op streams at
1 elem/lane/cycle (>=5 slices cannot pack into the 2x perf mode) and the
floor is 8 fused-DVE passes per x-element. The 2-pass clipped prologue
rs = k1*relu(2 - a) (a = |x - c_i|/h) is split to balance measured
per-pass HW rates (DVE custom 0.93 ns/elem, DVE stock tensor_scalar in
the fp16 4x_2p perf mode 0.23 ns/elem, ACT 0.73 ns/elem; GpSimd has no
max/min/abs ops and cannot help):

  - _TSP_CH (5 ch): ACT abar = Abs(k1/h*x - k1*c_i/h)   [= k1*a, fp16]
                    DVE  t = (abar sub 2k1) min 0        [stock
                         tensor_scalar, fp16 4x perf mode; t = -rs]
                    DVE  BSPL_C3N(t)                     [8-slice custom op;
                         the sign of t folds away via min-instead-of-relu]
  - _ACT_CH (3 ch): ACT a = Abs(x/h - c_i/h); ACT rs = Relu(-k1*a + 2k1);
                    DVE BSPL_C3(rs).

Per x-element: DVE 8x0.93 + 5x0.23 = 8.6 ns, ACT 11x0.73 = 8.0 ns -- both
engines ~fully busy; DMA (4 MB in + 16 MB out per core) measured ~24 us
total, fully hidden.

Output is written in fp16 (tolerance 2e-2; full-chain rel err ~9e-4) and
CHANNEL-MAJOR: each DVE op writes its channel contiguously -- packed 2-byte
writes; strided fp16 SBUF writes are a ~4x real-HW penalty. DRAM layout is
[P, chunk, channel, F]; the host deinterleaves to (..., F, channel) and
upcasts. fp16 halves the dominant store traffic (32 MB -> 16 MB per core).

Sharding: batch-parallel, rows 128*c .. 128*c+127 on core c (8 cores).
"""

import numpy as np

_CACHE = {}

_K1 = float(6.0 ** (-1.0 / 3.0))        # k1^3 = 1/6
_K2 = float((2.0 / 3.0) ** (1.0 / 3.0))  # k2^3 = 2/3

_P = 128          # SBUF partitions = rows per core
_COLS = 8192      # row length
_NB = 8           # basis functions
_F = 2048         # free-dim chunk per DVE op / store DMA
_FA = 2048        # free-dim span per prologue op
_PURE_CH = ()                   # DVE-only (BSPL_P + BSPL_Q) — unused at the
                                # measured optimum, kept for re-balancing
_ACT_CH = (4, 5, 6, 7)          # Abs + Relu on ACT, then one DVE C3
_TSP_CH = (0, 1, 2, 3)          # Abs on ACT, clip via DVE tensor_scalar
                                # (fp16 4x perf mode), then one DVE C3N
_NCORES = 8


def _register_custom_ops():
    import concourse.dve_ops as dve_ops
    from concourse.dve_ops import DveOp
    from concourse.dve_spec import (
        Spec, Src0, Src1, C0, C1, C2, Zero, relu, sq, lower, AluOp, Bin, minn,
    )
    from concourse.dve_uop import DveOpSpec

    def ref_p(in0, in1, s0, s1, imm2):
        w = imm2 - np.abs(in0.astype(np.float32) - s0) * s1
        return (np.square(np.maximum(w, 0)) * w).astype(np.float32)

    def ref_q(in0, in1, s0, s1, imm2):
        w = imm2 - np.abs(in0.astype(np.float32) - s0) * s1
        return (in1 - np.square(np.maximum(w, 0)) * w).astype(np.float32)

    def ref_c3(in0, in1, s0, s1, imm2):
        rs = in0.astype(np.float32)
        p = np.square(rs) * rs
        w1 = rs * s0 - s1
        s = np.maximum(w1, 0)
        return (p - np.square(s) * w1).astype(np.float32)

    def body_p():
        w = C2 - Bin(AluOp.ABSOLUTE_DIFF, Src0, C0) * C1
        return sq(relu(w)) * w

    def body_q():
        w1 = C2 - Bin(AluOp.ABSOLUTE_DIFF, Src0, C0) * C1
        return Src1 - sq(relu(w1)) * w1

    def body_c3():
        p = sq(Src0) * Src0
        w1 = Src0 * C0 - C1
        return p - sq(relu(w1)) * w1

    # C3N consumes the NEGATED clipped prologue t = min(abar - 2k1, 0) = -rs
    # (producible by one stock tensor_scalar, which unlike the custom ops can
    # use the fp16 2x/4x DVE perf modes). Signs fold away in 8 ALU ops:
    # nw1 = (k2/k1)*t + k2 = -w1; min(nw1,0) = -relu(w1);
    # out = sq(-relu(w1))*nw1 - sq(t)*t = p - q.
    def ref_c3n(in0, in1, s0, s1, imm2):
        t = in0.astype(np.float32)
        nw1 = t * s0 + s1
        r1 = np.minimum(nw1, 0)
        qn = np.square(r1) * nw1
        pp = np.square(t) * t
        return (qn - pp).astype(np.float32)

    def body_c3n():
        nw1 = Src0 * C0 + C1
        r1 = minn(nw1, Zero)
        qn = sq(r1) * nw1
        pp = sq(Src0) * Src0
        return qn - pp

    def make(name, body, ref):
        spec = Spec(body=body, reference=ref)
        shas = {}
        for ver in ("v3", "v4"):
            shas[ver] = DveOpSpec(name=name, uops=lower(spec, ver=ver)).sha(ver)
        return DveOp(name, spec, subdim=False, uops_sha=shas)

    ops = {}
    for name, body, ref in (
        ("BSPL_P", body_p(), ref_p),
        ("BSPL_Q", body_q(), ref_q),
        ("BSPL_C3", body_c3(), ref_c3),
        ("BSPL_C3N", body_c3n(), ref_c3n),
    ):
        existing = {op.name: op for op in dve_ops.OPS}
        if name in existing:
            ops[name] = existing[name]
            continue
        op = make(name, body, ref)
        dve_ops.OPS.append(op)
        dve_ops.CUSTOM_DVE_SPECS[op.name] = op.spec
        row = max(dve_ops._SUB_OPCODE_FOR_NAME.values()) + 1
        assert row < 0x20
        dve_ops._SUB_OPCODE_FOR_NAME[op.name] = row
        ops[name] = op
    return (ops["BSPL_P"], ops["BSPL_Q"], ops["BSPL_C3"], ops["BSPL_C3N"])


def _build(knot0: float, h: float, passes: int = 1):
    import concourse.bacc as bacc
    import concourse.mybir as mybir
    from concourse import tile

    AF = mybir.ActivationFunctionType
    ALU = mybir.AluOpType
    bspl_p, bspl_q, bspl_c3, bspl_c3n = _register_custom_ops()

    nc = bacc.Bacc("TRN2", target_bir_lowering=False, debug=False,
                   num_devices=_NCORES)
    x_ext = nc.declare_dram_parameter("x", [_P, _COLS], mybir.dt.float32,
                                      isOutput=False)
    out_ext = nc.declare_dram_parameter("out", [_P, _COLS * _NB],
                                        mybir.dt.float16, isOutput=True)

    with tile.TileContext(nc) as tc:
        with tc.tile_pool(name="xin", bufs=2) as xin, \
             tc.tile_pool(name="ilp", bufs=2) as ilp, \
             tc.tile_pool(name="wk", bufs=2) as wk, \
             tc.tile_pool(name="dp", bufs=1) as dpool, \
             tc.tile_pool(name="aact", bufs=1) as aact, \
             tc.tile_pool(name="rp", bufs=2) as rpool, \
             tc.tile_pool(name="cst", bufs=1) as cst:
            # ACT's float bias operands must live in SBUF as [P, 1] const APs.
            cvals = sorted({-(knot0 + (i + 2) * h) / h for i in _ACT_CH}
                           | {-(knot0 + (i + 2) * h) * _K1 / h
                              for i in _TSP_CH}
                           | {2.0 * _K1})
            for v in cvals:
                t = cst.tile([_P, 1], mybir.dt.float32, tag=f"c{v}")
                nc.vector.memset(t[:], float(v))
                nc.const_aps.aps[(mybir.dt.float32, float(v))] = t
            for rep in range(passes):
                # Software-pipelined emission: issue span s+1's load +
                # prologue before span s's DVE/store work so the scheduler
                # overlaps prologues with the previous span's compute.
                def _load_prologue(s):
                    xs = xin.tile([_P, _FA], mybir.dt.float32, tag="x")
                    for half in range(_FA // _F):
                        nc.sync.dma_start(
                            xs[:, half * _F:(half + 1) * _F],
                            x_ext[:, s * _FA + half * _F:
                                  s * _FA + (half + 1) * _F])
                    rss = {}
                    for i in _TSP_CH + _ACT_CH:
                        c_i = knot0 + (i + 2) * h
                        if i in _TSP_CH:
                            # ACT folds the k1/h scale into the Abs, then one
                            # stock DVE tensor_scalar (fp16 -> 2x/4x perf
                            # mode) produces t = min(abar - 2k1, 0) = -rs,
                            # consumed by BSPL_C3N.
                            ab = dpool.tile([_P, _FA], mybir.dt.float16,
                                            tag=f"d{i}")
                            nc.scalar.activation(
                                ab[:], xs[:], AF.Abs,
                                bias=-c_i * _K1 / h, scale=_K1 / h)
                            rs = rpool.tile([_P, _FA], mybir.dt.float16,
                                            tag=f"rs{i}")
                            nc.vector.tensor_scalar(
                                rs[:], ab[:], 2.0 * _K1, 0.0,
                                ALU.subtract, ALU.min)
                        else:
                            a = aact.tile([_P, _FA], mybir.dt.float16,
                                          tag=f"a{i}")
                            nc.scalar.activation(a[:], xs[:], AF.Abs,
                                                 bias=-c_i / h, scale=1.0 / h)
                            rs = rpool.tile([_P, _FA], mybir.dt.float16,
                                            tag=f"rs{i}")
                            nc.scalar.activation(rs[:], a[:], AF.Relu,
                                                 bias=2.0 * _K1, scale=-_K1)
                        rss[i] = rs
                    return xs, rss
                nspan = _COLS // _FA
                pending = _load_prologue(0)
                for s in range(nspan):
                    xs, rss = pending
                    if s + 1 < nspan:
                        pending = _load_prologue(s + 1)
                    for half in range(_FA // _F):
                        c = s * (_FA // _F) + half
                        lo, hi = half * _F, (half + 1) * _F
                        # Channel-major tile: each DVE op writes its channel
                        # contiguously (packed fp16). Host deinterleaves.
                        il = ilp.tile([_P, _NB, _F], mybir.dt.float16,
                                      tag="il")
                        for i in range(_NB):
                            if i in _PURE_CH:
                                c_i = knot0 + (i + 2) * h
                                p = wk.tile([_P, _F], mybir.dt.float32,
                                            tag="p")
                                nc.vector._custom_dve(
                                    bspl_p, out=p[:], in0=xs[:, lo:hi],
                                    s0=c_i, s1=_K1 / h, imm2=2.0 * _K1)
                                nc.vector._custom_dve(
                                    bspl_q, out=il[:, i, :],
                                    in0=xs[:, lo:hi], in1=p[:],
                                    s0=c_i, s1=_K2 / h, imm2=_K2)
                            elif i in _TSP_CH:
                                nc.vector._custom_dve(
                                    bspl_c3n, out=il[:, i, :],
                                    in0=rss[i][:, lo:hi],
                                    s0=_K2 / _K1, s1=_K2)
                            else:
                                nc.vector._custom_dve(
                                    bspl_c3, out=il[:, i, :],
                                    in0=rss[i][:, lo:hi],
                                    s0=_K2 / _K1, s1=_K2)
                        nc.sync.dma_start(
                            out_ext[:, c * _F * _NB:(c + 1) * _F * _NB],
                            il.rearrange("p e f -> p (e f)"))

    nc.compile()
    return nc


def _numpy_fallback(x, knots):
    """Cox-de Boor on host — only used if knots are not uniform (the
    reference always generates uniform knots; this is a safety net)."""
    te = x[..., None]
    B = ((knots[:-1] <= te) & (te < knots[1:])).astype(np.float32)
    nk = len(knots)
    for k in range(1, 4):
        n = nk - k - 1
        ld = knots[k:k + n] - knots[:n]
        rd = knots[k + 1:k + 1 + n] - knots[1:1 + n]
        left = np.where(ld != 0, (te - knots[:n]) / ld, 0.0) * B[..., :n]
        right = (np.where(rd != 0, (knots[k + 1:k + 1 + n] - te) / rd, 0.0)
                 * B[..., 1:n + 1])
        B = (left + right).astype(np.float32)
    return B[..., :_NB]


def kernel(x: np.ndarray, knots: np.ndarray | None = None, **_ignored):
    from concourse.bass_utils import run_bass_kernel_spmd

    x = np.ascontiguousarray(np.asarray(x, dtype=np.float32))
    if knots is None:
        knots = np.linspace(-2.0, 2.0, 12, dtype=np.float32)
    knots = np.asarray(knots, dtype=np.float32)
    assert x.shape == (_P * _NCORES, _COLS), x.shape
    knot0 = float(knots[0])
    h = float(knots[-1] - knots[0]) / (len(knots) - 1)
    if not np.allclose(np.diff(knots), h, rtol=1e-5, atol=1e-6):
        return _numpy_fallback(x, knots)

    key = (knot0, h)
    if key not in _CACHE:
        _CACHE[key] = _build(knot0, h)
    nc = _CACHE[key]

    in_maps = [{"x": x[c * _P:(c + 1) * _P]} for c in range(_NCORES)]
    res = run_bass_kernel_spmd(nc, in_maps, list(range(_NCORES)))
    out = np.empty((_P * _NCORES, _COLS, _NB), dtype=np.float32)
    nchunk = _COLS // _F
    for c in range(_NCORES):
        # Device layout: [P, chunk, channel, F] — deinterleave on host.
        r = res.results[c]["out"].reshape(_P, nchunk, _NB, _F)
        out[c * _P:(c + 1) * _P] = r.transpose(0, 1, 3, 2).reshape(
            _P, _COLS, _NB).astype(np.float32)
    return out


# revision 29
# speedup vs baseline: 1.2684x; 1.2684x over previous
"""Trainium2 Bass kernel for degree-3 uniform B-spline basis evaluation.

Problem: x (1024, 8192) fp32, knots = linspace(-2, 2, 12) -> out (1024, 8192, 8)
where out[..., i] is the i-th cubic B-spline basis function (Cox-de Boor).

Math. With uniform knots (spacing h), basis i is a shifted cardinal cubic
B-spline: out_i(x) = C((x - knots[0])/h - i), C supported on [0, 4). Writing
a = |(x - knots[0])/h - i - 2| (distance to the support center), C reflects to

    C = relu(2 - a)^3 / 6  -  (2/3) * relu(1 - a)^3

which is numerically clean and returns exact zeros outside the support.

Kernel design (measured-HW engine balance at the DVE write floor):

Every output element must be produced by one DVE write: the fused cubic
needs 8 ALU slices, the DVE pipeline max, so the terminal op streams at
1 elem/lane/cycle (>=5 slices cannot pack into the 2x perf mode) and the
floor is 8 fused-DVE passes per x-element. The 2-pass clipped prologue
rs = k1*relu(2 - a) (a = |x - c_i|/h) is split to balance measured
per-pass HW rates (DVE custom 0.93 ns/elem, DVE stock tensor_scalar in
the fp16 4x_2p perf mode 0.23 ns/elem, ACT 0.73 ns/elem; GpSimd has no
max/min/abs ops and cannot help):

  - _TSP_CH (5 ch): ACT abar = Abs(k1/h*x - k1*c_i/h)   [= k1*a, fp16]
                    DVE  t = (abar sub 2k1) min 0        [stock
                         tensor_scalar, fp16 4x perf mode; t = -rs]
                    DVE  BSPL_C3N(t)                     [8-slice custom op;
                         the sign of t folds away via min-instead-of-relu]
  - _ACT_CH (3 ch): ACT a = Abs(x/h - c_i/h); ACT rs = Relu(-k1*a + 2k1);
                    DVE BSPL_C3(rs).

Per x-element: DVE 8x0.93 + 5x0.23 = 8.6 ns, ACT 11x0.73 = 8.0 ns -- both
engines ~fully busy; DMA (4 MB in + 16 MB out per core) measured ~24 us
total, fully hidden.

Output is written in fp16 (tolerance 2e-2; full-chain rel err ~9e-4) and
CHANNEL-MAJOR: each DVE op writes its channel contiguously -- packed 2-byte
writes; strided fp16 SBUF writes are a ~4x real-HW penalty. DRAM layout is
[P, chunk, channel, F]; the host deinterleaves to (..., F, channel) and
upcasts. fp16 halves the dominant store traffic (32 MB -> 16 MB per core).

Sharding: batch-parallel, rows 128*c .. 128*c+127 on core c (8 cores).
"""

import numpy as np

_CACHE = {}

_K1 = float(6.0 ** (-1.0 / 3.0))        # k1^3 = 1/6
_K2 = float((2.0 / 3.0) ** (1.0 / 3.0))  # k2^3 = 2/3

_P = 128          # SBUF partitions = rows per core
_COLS = 8192      # row length
_NB = 8           # basis functions
_F = 2048         # free-dim chunk per DVE op / store DMA
_FA = 2048        # free-dim span per prologue op
_PURE_CH = ()                   # DVE-only (BSPL_P + BSPL_Q) — unused at the
                                # measured optimum, kept for re-balancing
_ACT_CH = (5, 6, 7)             # Abs + Relu on ACT, then one DVE C3
_TSP_CH = (0, 1, 2, 3, 4)       # Abs on ACT, clip via DVE tensor_scalar
                                # (fp16 4x perf mode), then one DVE C3N
_NCORES = 8


def _register_custom_ops():
    import concourse.dve_ops as dve_ops
    from concourse.dve_ops import DveOp
    from concourse.dve_spec import (
        Spec, Src0, Src1, C0, C1, C2, Zero, relu, sq, lower, AluOp, Bin, minn,
    )
    from concourse.dve_uop import DveOpSpec

    def ref_p(in0, in1, s0, s1, imm2):
        w = imm2 - np.abs(in0.astype(np.float32) - s0) * s1
        return (np.square(np.maximum(w, 0)) * w).astype(np.float32)

    def ref_q(in0, in1, s0, s1, imm2):
        w = imm2 - np.abs(in0.astype(np.float32) - s0) * s1
        return (in1 - np.square(np.maximum(w, 0)) * w).astype(np.float32)

    def ref_c3(in0, in1, s0, s1, imm2):
        rs = in0.astype(np.float32)
        p = np.square(rs) * rs
        w1 = rs * s0 - s1
        s = np.maximum(w1, 0)
        return (p - np.square(s) * w1).astype(np.float32)

    def body_p():
        w = C2 - Bin(AluOp.ABSOLUTE_DIFF, Src0, C0) * C1
        return sq(relu(w)) * w

    def body_q():
        w1 = C2 - Bin(AluOp.ABSOLUTE_DIFF, Src0, C0) * C1
        return Src1 - sq(relu(w1)) * w1

    def body_c3():
        p = sq(Src0) * Src0
        w1 = Src0 * C0 - C1
        return p - sq(relu(w1)) * w1

    # C3N consumes the NEGATED clipped prologue t = min(abar - 2k1, 0) = -rs
    # (producible by one stock tensor_scalar, which unlike the custom ops can
    # use the fp16 2x/4x DVE perf modes). Signs fold away in 8 ALU ops:
    # nw1 = (k2/k1)*t + k2 = -w1; min(nw1,0) = -relu(w1);
    # out = sq(-relu(w1))*nw1 - sq(t)*t = p - q.
    def ref_c3n(in0, in1, s0, s1, imm2):
        t = in0.astype(np.float32)
        nw1 = t * s0 + s1
        r1 = np.minimum(nw1, 0)
        qn = np.square(r1) * nw1
        pp = np.square(t) * t
        return (qn - pp).astype(np.float32)

    def body_c3n():
        nw1 = Src0 * C0 + C1
        r1 = minn(nw1, Zero)
        qn = sq(r1) * nw1
        pp = sq(Src0) * Src0
        return qn - pp

    def make(name, body, ref):
        spec = Spec(body=body, reference=ref)
        shas = {}
        for ver in ("v3", "v4"):
            shas[ver] = DveOpSpec(name=name, uops=lower(spec, ver=ver)).sha(ver)
        return DveOp(name, spec, subdim=False, uops_sha=shas)

    ops = {}
    for name, body, ref in (
        ("BSPL_P", body_p(), ref_p),
        ("BSPL_Q", body_q(), ref_q),
        ("BSPL_C3", body_c3(), ref_c3),
        ("BSPL_C3N", body_c3n(), ref_c3n),
    ):
        existing = {op.name: op for op in dve_ops.OPS}
        if name in existing:
            ops[name] = existing[name]
            continue
        op = make(name, body, ref)
        dve_ops.OPS.append(op)
        dve_ops.CUSTOM_DVE_SPECS[op.name] = op.spec
        row = max(dve_ops._SUB_OPCODE_FOR_NAME.values()) + 1
        assert row < 0x20
        dve_ops._SUB_OPCODE_FOR_NAME[op.name] = row
        ops[name] = op
    return (ops["BSPL_P"], ops["BSPL_Q"], ops["BSPL_C3"], ops["BSPL_C3N"])


def _build(knot0: float, h: float, passes: int = 1):
    import concourse.bacc as bacc
    import concourse.mybir as mybir
    from concourse import tile

    AF = mybir.ActivationFunctionType
    ALU = mybir.AluOpType
    bspl_p, bspl_q, bspl_c3, bspl_c3n = _register_custom_ops()

    nc = bacc.Bacc("TRN2", target_bir_lowering=False, debug=False,
                   num_devices=_NCORES)
    x_ext = nc.declare_dram_parameter("x", [_P, _COLS], mybir.dt.float32,
                                      isOutput=False)
    out_ext = nc.declare_dram_parameter("out", [_P, _COLS * _NB],
                                        mybir.dt.float16, isOutput=True)

    with tile.TileContext(nc) as tc:
        with tc.tile_pool(name="xin", bufs=2) as xin, \
             tc.tile_pool(name="ilp", bufs=2) as ilp, \
             tc.tile_pool(name="wk", bufs=2) as wk, \
             tc.tile_pool(name="dp", bufs=1) as dpool, \
             tc.tile_pool(name="aact", bufs=1) as aact, \
             tc.tile_pool(name="rp", bufs=2) as rpool, \
             tc.tile_pool(name="cst", bufs=1) as cst:
            # ACT's float bias operands must live in SBUF as [P, 1] const APs.
            cvals = sorted({-(knot0 + (i + 2) * h) / h for i in _ACT_CH}
                           | {-(knot0 + (i + 2) * h) * _K1 / h
                              for i in _TSP_CH}
                           | {2.0 * _K1})
            for v in cvals:
                t = cst.tile([_P, 1], mybir.dt.float32, tag=f"c{v}")
                nc.vector.memset(t[:], float(v))
                nc.const_aps.aps[(mybir.dt.float32, float(v))] = t
            for rep in range(passes):
                # Software-pipelined emission: issue span s+1's load +
                # prologue before span s's DVE/store work so the scheduler
                # overlaps prologues with the previous span's compute.
                def _load_prologue(s):
                    xs = xin.tile([_P, _FA], mybir.dt.float32, tag="x")
                    for half in range(_FA // _F):
                        nc.sync.dma_start(
                            xs[:, half * _F:(half + 1) * _F],
                            x_ext[:, s * _FA + half * _F:
                                  s * _FA + (half + 1) * _F])
                    rss = {}
                    for i in _TSP_CH + _ACT_CH:
                        c_i = knot0 + (i + 2) * h
                        if i in _TSP_CH:
                            # ACT folds the k1/h scale into the Abs, then one
                            # stock DVE tensor_scalar (fp16 -> 2x/4x perf
                            # mode) produces t = min(abar - 2k1, 0) = -rs,
                            # consumed by BSPL_C3N.
                            ab = dpool.tile([_P, _FA], mybir.dt.float16,
                                            tag=f"d{i}")
                            nc.scalar.activation(
                                ab[:], xs[:], AF.Abs,
                                bias=-c_i * _K1 / h, scale=_K1 / h)
                            rs = rpool.tile([_P, _FA], mybir.dt.float16,
                                            tag=f"rs{i}")
                            nc.vector.tensor_scalar(
                                rs[:], ab[:], 2.0 * _K1, 0.0,
                                ALU.subtract, ALU.min)
                        else:
                            a = aact.tile([_P, _FA], mybir.dt.float16,
                                          tag=f"a{i}")
                            nc.scalar.activation(a[:], xs[:], AF.Abs,
                                                 bias=-c_i / h, scale=1.0 / h)
                            rs = rpool.tile([_P, _FA], mybir.dt.float16,
                                            tag=f"rs{i}")
                            nc.scalar.activation(rs[:], a[:], AF.Relu,
                                                 bias=2.0 * _K1, scale=-_K1)
                        rss[i] = rs
                    return xs, rss
                nspan = _COLS // _FA
                pending = _load_prologue(0)
                for s in range(nspan):
                    xs, rss = pending
                    if s + 1 < nspan:
                        pending = _load_prologue(s + 1)
                    for half in range(_FA // _F):
                        c = s * (_FA // _F) + half
                        lo, hi = half * _F, (half + 1) * _F
                        # Channel-major tile: each DVE op writes its channel
                        # contiguously (packed fp16). Host deinterleaves.
                        il = ilp.tile([_P, _NB, _F], mybir.dt.float16,
                                      tag="il")
                        for i in range(_NB):
                            if i in _PURE_CH:
                                c_i = knot0 + (i + 2) * h
                                p = wk.tile([_P, _F], mybir.dt.float32,
                                            tag="p")
                                nc.vector._custom_dve(
                                    bspl_p, out=p[:], in0=xs[:, lo:hi],
                                    s0=c_i, s1=_K1 / h, imm2=2.0 * _K1)
                                nc.vector._custom_dve(
                                    bspl_q, out=il[:, i, :],
                                    in0=xs[:, lo:hi], in1=p[:],
                                    s0=c_i, s1=_K2 / h, imm2=_K2)
                            elif i in _TSP_CH:
                                nc.vector._custom_dve(
                                    bspl_c3n, out=il[:, i, :],
                                    in0=rss[i][:, lo:hi],
                                    s0=_K2 / _K1, s1=_K2)
                            else:
                                nc.vector._custom_dve(
                                    bspl_c3, out=il[:, i, :],
                                    in0=rss[i][:, lo:hi],
                                    s0=_K2 / _K1, s1=_K2)
                        nc.sync.dma_start(
                            out_ext[:, c * _F * _NB:(c + 1) * _F * _NB],
                            il.rearrange("p e f -> p (e f)"))

    nc.compile()
    return nc


def _numpy_fallback(x, knots):
    """Cox-de Boor on host — only used if knots are not uniform (the
    reference always generates uniform knots; this is a safety net)."""
    te = x[..., None]
    B = ((knots[:-1] <= te) & (te < knots[1:])).astype(np.float32)
    nk = len(knots)
    for k in range(1, 4):
        n = nk - k - 1
        ld = knots[k:k + n] - knots[:n]
        rd = knots[k + 1:k + 1 + n] - knots[1:1 + n]
        left = np.where(ld != 0, (te - knots[:n]) / ld, 0.0) * B[..., :n]
        right = (np.where(rd != 0, (knots[k + 1:k + 1 + n] - te) / rd, 0.0)
                 * B[..., 1:n + 1])
        B = (left + right).astype(np.float32)
    return B[..., :_NB]


def kernel(x: np.ndarray, knots: np.ndarray | None = None, **_ignored):
    from concourse.bass_utils import run_bass_kernel_spmd

    x = np.ascontiguousarray(np.asarray(x, dtype=np.float32))
    if knots is None:
        knots = np.linspace(-2.0, 2.0, 12, dtype=np.float32)
    knots = np.asarray(knots, dtype=np.float32)
    assert x.shape == (_P * _NCORES, _COLS), x.shape
    knot0 = float(knots[0])
    h = float(knots[-1] - knots[0]) / (len(knots) - 1)
    if not np.allclose(np.diff(knots), h, rtol=1e-5, atol=1e-6):
        return _numpy_fallback(x, knots)

    key = (knot0, h)
    if key not in _CACHE:
        _CACHE[key] = _build(knot0, h)
    nc = _CACHE[key]

    in_maps = [{"x": x[c * _P:(c + 1) * _P]} for c in range(_NCORES)]
    res = run_bass_kernel_spmd(nc, in_maps, list(range(_NCORES)))
    out = np.empty((_P * _NCORES, _COLS, _NB), dtype=np.float32)
    nchunk = _COLS // _F
    for c in range(_NCORES):
        # Device layout: [P, chunk, channel, F] — deinterleave on host.
        r = res.results[c]["out"].reshape(_P, nchunk, _NB, _F)
        out[c * _P:(c + 1) * _P] = r.transpose(0, 1, 3, 2).reshape(
            _P, _COLS, _NB).astype(np.float32)
    return out
